# revision 55
# baseline (speedup 1.0000x reference)
"""Two-layer GCN (PyG GCNConv x2 + ReLU) on 8 Trainium2 NeuronCores.

Strategy (no redundant compute):
  - Nodes are packed into 128-slot tiles balanced by in-degree (LPT) and
    sharded across cores (49 tiles/core). Each core computes
    g1 = dinv * (x @ W1) ONLY for its own 6272 nodes, then two chunked
    AllGathers (lo: tiles 0..cA-1 of every core, hi: the rest; lo fires
    early, mid-phase) build the full layer-1 message table in fp8e4m3.
    A bf16 copy (g1sb) serves the self-loop term, which is too
    precision-sensitive for fp8 (low-degree nodes are dominated by it).
  - Gather tables are stored f32-"packed" (bytes reinterpreted): the tile
    scheduler prices a gather by OUTPUT ELEMENT COUNT, so wide elements are
    proportionally cheaper. Compute reads bf16/fp8 bitcast views.
  - Two consecutive tiles share each lo/hi dma_gather (halves gather count).
  - Aggregation: one-hot (is_equal) matmuls accumulate messages in PSUM.
    Layer-1 blocks go in fp8 DoubleRow PAIRS (two 128-message blocks per
    matmul at half cost). Bias enters PSUM via a diag(sqrt(deg)) @ bias_rep
    matmul so the Act engine fuses the dinv scale (+ ReLU) in one op.
  - Layer 2: g2 = dinv * (a1 @ W2) fused per tile (bf16, accuracy-critical),
    exchanged with two more chunked AllGathers, aggregated with bf16
    one-hot matmuls.
  - Output written bf16, upcast to f32 on host.
"""

import math
import heapq

import numpy as np
import ml_dtypes

from concourse import bacc, mybir
from concourse.tile import TileContext
from concourse.bass_utils import run_bass_kernel_spmd

BF16 = ml_dtypes.bfloat16
N_CORES = 8

# cost-model predicted makespan (ns) of the last _build_nc, for diagnostics
LAST_PREDICTED_NS = None


def _capture_schedule(tc_cls):
    orig = tc_cls.schedule_and_allocate

    def patched(self, validate_deps=False):
        global LAST_PREDICTED_NS
        r = orig(self, validate_deps)
        try:
            LAST_PREDICTED_NS = int(r[1].time)
        except Exception:
            pass
        return r

    if getattr(tc_cls, "_gnn_patched", False):
        return
    tc_cls.schedule_and_allocate = patched
    tc_cls._gnn_patched = True


_capture_schedule(TileContext)

# Full-problem config. Tests may monkeypatch _CFG before calling kernel().
_CFG = dict(
    N=50000,
    E=800000,
    IN=768,
    HID=512,
    OUT=256,
    T=49,  # tiles per core
)


def _pack_idx(idx_linear):
    """[n16] int (n16 % 16 == 0) -> [128, n16//16] int16 in dma_gather layout."""
    num = idx_linear.shape[0]
    a = idx_linear.reshape(num // 16, 16).T.astype(np.int16)
    return np.tile(a, (8, 1))


def _build_nc(cfg, meta):
    IN, HID, OUT = cfg["IN"], cfg["HID"], cfg["OUT"]
    T = cfg["T"]
    PC = T * 128
    NK1 = IN // 128
    NK2 = HID // 128
    cA, cB = meta["ch_tiles"]  # 25, 24
    RA, RB = N_CORES * cA * 128, N_CORES * cB * 128
    KL, KH = meta["KL"], meta["KH"]  # per-tile block counts
    pairs = meta["pairs"]  # per gather-pair: (tiles, Llo, Lhi, KLp, KHp)
    NBMAX = meta["NBMAX"]
    C8PMAX = meta["C8PMAX"]

    f32 = mybir.dt.float32
    bf = mybir.dt.bfloat16
    f8 = mybir.dt.float8e4
    i16 = mybir.dt.int16

    PK1 = HID // 4  # f32-packed row width, layer-1 fp8 table
    PKB = HID // 2  # f32-packed row width, layer-1 bf16 self-loop table
    PK2 = OUT // 2  # f32-packed row width, layer-2 bf16 table
    nc = bacc.Bacc(None, target_bir_lowering=False, debug=False)
    xT_p = nc.declare_dram_parameter("xT", [IN, PC], bf, isOutput=False)
    w1_p = nc.declare_dram_parameter("w1p", [128, NK1 * HID], bf, isOutput=False)
    w2_p = nc.declare_dram_parameter("w2p", [128, NK2 * OUT], bf, isOutput=False)
    b1_p = nc.declare_dram_parameter("b1r", [128, HID], bf, isOutput=False)
    b2_p = nc.declare_dram_parameter("b2r", [128, OUT], bf, isOutput=False)
    iota_p = nc.declare_dram_parameter("iota", [128, 128], bf, isOutput=False)
    ident_p = nc.declare_dram_parameter("identb", [128, 128], bf, isOutput=False)
    dinv_p = nc.declare_dram_parameter("dinvT", [128, T], f32, isOutput=False)
    dsq_p = nc.declare_dram_parameter("dsqT", [128, T], f32, isOutput=False)
    idx_p = nc.declare_dram_parameter("idxt", [len(pairs) * 128, C8PMAX], i16, isOutput=False)
    dl_p = nc.declare_dram_parameter("dlt", [T * 128, NBMAX], f32, isOutput=False)
    out_p = nc.declare_dram_parameter("out", [PC, OUT], bf, isOutput=True)

    with TileContext(nc) as tc:
        with (
            tc.tile_pool(name="const", bufs=1) as cpool,
            tc.tile_pool(name="work", bufs=2) as wpool,
            tc.tile_pool(name="psum", bufs=2, space="PSUM") as ppool,
            tc.tile_pool(name="dram", bufs=1, space="DRAM") as dpool,
        ):
            # ---- internal DRAM (gather tables packed as f32: the tile
            # scheduler prices gathers by output element count, so wide
            # elements are cheaper; layer-1 messages are fp8 (halves elems
            # again), the self-loop reads a separate bf16 copy, layer 2 is
            # bf16. int64 packing would be wider still but the HW gather
            # mangles 8B elems) ----
            g1s = dpool.tile([PC, PK1], f32, name="g1s")
            g1sb = dpool.tile([PC, PKB], f32, name="g1sb")
            g2s = dpool.tile([PC, PK2], f32, name="g2s")
            g1fA = dpool.tile([RA, PK1], f32, name="g1fA", addr_space="Shared")
            g1fB = dpool.tile([RB, PK1], f32, name="g1fB", addr_space="Shared")
            g2fA = dpool.tile([RA, PK2], f32, name="g2fA", addr_space="Shared")
            g2fB = dpool.tile([RB, PK2], f32, name="g2fB", addr_space="Shared")

            # ---- constants ----
            w1sb = cpool.tile([128, NK1 * HID], bf, name="w1sb")
            nc.sync.dma_start(out=w1sb[:, :], in_=w1_p[:, :])
            w2sb = cpool.tile([128, NK2 * OUT], bf, name="w2sb")
            nc.sync.dma_start(out=w2sb[:, :], in_=w2_p[:, :])
            b1sb = cpool.tile([128, HID], bf, name="b1sb")
            nc.sync.dma_start(out=b1sb[:, :], in_=b1_p[:, :])
            b2sb = cpool.tile([128, OUT], bf, name="b2sb")
            nc.sync.dma_start(out=b2sb[:, :], in_=b2_p[:, :])
            iot = cpool.tile([128, 128], bf, name="iot")
            nc.sync.dma_start(out=iot[:, :], in_=iota_p[:, :])
            idn = cpool.tile([128, 128], bf, name="idn")
            nc.sync.dma_start(out=idn[:, :], in_=ident_p[:, :])
            dnv = cpool.tile([128, T], f32, name="dnv")
            nc.sync.dma_start(out=dnv[:, :], in_=dinv_p[:, :])
            dsq = cpool.tile([128, T], f32, name="dsq")
            nc.sync.dma_start(out=dsq[:, :], in_=dsq_p[:, :])

            # gather dst buffers (two tiles' blocks per gather pair),
            # f32-packed; matmuls read per-block fp8/bf16 bitcast views
            def msg_tile(tag, width, bufs=3):
                return wpool.tile(
                    [128, 2 * NBMAX, width], f32, tag=tag, bufs=bufs, name=tag
                )

            diags = []  # diag(sqrt(deg)) per tile, for bias matmul

            def paired_gather(mt, KLp, KHp, srcA, srcB, ix, c8lo, c8hi, Llo, Lhi, width):
                if Llo:
                    nc.gpsimd.dma_gather(
                        mt[:, :KLp, :], srcA[:, :], ix[:, :c8lo],
                        Llo, Llo, width, single_packet=False,
                    )
                if Lhi:
                    nc.gpsimd.dma_gather(
                        mt[:, KLp : KLp + KHp, :], srcB[:, :], ix[:, c8lo : c8lo + c8hi],
                        Lhi, Lhi, width, single_packet=False,
                    )

            # ---- phase 1: g1 = dinv * (x @ W1) for OWN nodes only ----
            CH = min(5, T)  # node tiles per x-chunk
            for ch0 in range(0, T, CH):
                chn = min(CH, T - ch0)
                xk = []
                for k in range(NK1):
                    xt = wpool.tile([128, CH * 128], bf, tag=f"xk{k}", bufs=2)
                    nc.sync.dma_start(
                        out=xt[:, :chn * 128],
                        in_=xT_p[k * 128 : (k + 1) * 128, ch0 * 128 : (ch0 + chn) * 128],
                    )
                    xk.append(xt)
                for sub in range(chn):
                    t = ch0 + sub
                    ps = ppool.tile([128, HID], f32, tag="p1", bufs=2)
                    for k in range(NK1):
                        nc.tensor.matmul(
                            ps[:, :],
                            xk[k][:, sub * 128 : (sub + 1) * 128],
                            w1sb[:, k * HID : (k + 1) * HID],
                            start=(k == 0),
                            stop=(k == NK1 - 1),
                        )
                    ge = wpool.tile([128, HID], bf, tag="ge", bufs=2)
                    nc.scalar.activation(
                        ge[:, :], ps[:, :], mybir.ActivationFunctionType.Copy,
                        scale=dnv[:, t : t + 1],
                    )
                    nc.sync.dma_start(
                        out=g1sb[t * 128 : (t + 1) * 128, :],
                        in_=ge[:, :].bitcast(f32),
                    )
                    ge8 = wpool.tile([128, HID], f8, tag="ge8", bufs=2)
                    nc.scalar.activation(
                        ge8[:, :], ps[:, :], mybir.ActivationFunctionType.Copy,
                        scale=dnv[:, t : t + 1],
                    )
                    nc.sync.dma_start(
                        out=g1s[t * 128 : (t + 1) * 128, :],
                        in_=ge8[:, :].bitcast(f32),
                    )
                    if t == cA - 1:
                        nc.gpsimd.collective_compute(
                            "AllGather",
                            mybir.AluOpType.bypass,
                            ins=[g1s[0 : cA * 128, :].opt()],
                            outs=[g1fA[:, :].opt()],
                            replica_groups=[list(range(N_CORES))],
                        )
            nc.gpsimd.collective_compute(
                "AllGather",
                mybir.AluOpType.bypass,
                ins=[g1s[cA * 128 :, :].opt()],
                outs=[g1fB[:, :].opt()],
                replica_groups=[list(range(N_CORES))],
            )

            # ---- phase 2: layer-1 aggregation + fused mm2 ----
            for p, (ptiles, Llo, Lhi, KLp, KHp) in enumerate(pairs):
                c8lo, c8hi = Llo // 16, Lhi // 16
                ix = wpool.tile([128, C8PMAX], i16, tag="ix", bufs=3)
                nc.sync.dma_start(
                    out=ix[:, : c8lo + c8hi],
                    in_=idx_p[p * 128 : (p + 1) * 128, : c8lo + c8hi],
                )
                m1 = msg_tile("m1", PK1, bufs=4)
                paired_gather(m1, KLp, KHp, g1fA, g1fB, ix, c8lo, c8hi, Llo, Lhi, PK1)

                off_lo = 0
                off_hi = KLp
                for t in ptiles:
                    kl, kh = KL[t], KH[t]
                    nb = kl + kh
                    dl = wpool.tile([128, NBMAX], f32, tag="dl", bufs=3)
                    nc.sync.dma_start(
                        out=dl[:, :nb], in_=dl_p[t * 128 : (t + 1) * 128, :nb]
                    )
                    gs = wpool.tile([128, PKB], f32, tag="gs", bufs=3)
                    nc.sync.dma_start(
                        out=gs[:, :], in_=g1sb[t * 128 : (t + 1) * 128, :]
                    )

                    diag = cpool.tile([128, 128], bf, name=f"dg{t}")
                    nc.vector.tensor_scalar(
                        diag[:, :], idn[:, :], dsq[:, t : t + 1], None, mybir.AluOpType.mult
                    )
                    diags.append(diag)

                    ps = ppool.tile([128, HID], f32, tag="p1", bufs=2)
                    nc.tensor.matmul(ps[:, :], diag[:, :], b1sb[:, :], start=True, stop=False)
                    # fp8 DoubleRow: two message blocks per matmul (half cost)
                    for off, cnt, dlb in ((off_lo, kl, 0), (off_hi, kh, kl)):
                        b = 0
                        while b + 1 < cnt:
                            oh2 = wpool.tile([128, 2, 128], f8, tag="oh2", bufs=8)
                            nc.vector.tensor_scalar(
                                oh2[:, 0, :], iot[:, :], dl[:, dlb + b : dlb + b + 1],
                                None, mybir.AluOpType.is_equal,
                            )
                            nc.vector.tensor_scalar(
                                oh2[:, 1, :], iot[:, :], dl[:, dlb + b + 1 : dlb + b + 2],
                                None, mybir.AluOpType.is_equal,
                            )
                            nc.tensor.matmul(
                                ps[:, :], oh2[:, :, :],
                                m1[:, off + b : off + b + 2, :].bitcast(f8),
                                start=False, stop=False,
                                perf_mode=mybir.MatmulPerfMode.DoubleRow,
                            )
                            b += 2
                        if b < cnt:
                            oh = wpool.tile([128, 128], bf, tag="oh", bufs=8)
                            nc.vector.tensor_scalar(
                                oh[:, :], iot[:, :], dl[:, dlb + b : dlb + b + 1],
                                None, mybir.AluOpType.is_equal,
                            )
                            nc.tensor.matmul(
                                ps[:, :], oh[:, :], m1[:, off + b, :].bitcast(f8),
                                start=False, stop=False,
                            )
                    # self-loop: own bf16 g1 rows read back from g1sb
                    nc.tensor.matmul(ps[:, :], idn[:, :], gs[:, :].bitcast(bf), start=False, stop=True)

                    a1 = wpool.tile([128, HID], bf, tag="a1", bufs=2)
                    nc.scalar.activation(
                        a1[:, :], ps[:, :], mybir.ActivationFunctionType.Relu,
                        scale=dnv[:, t : t + 1],
                    )

                    ps2 = ppool.tile([128, OUT], f32, tag="p2", bufs=2)
                    for k in range(NK2):
                        pT = ppool.tile([128, 128], bf, tag="pT", bufs=2)
                        nc.tensor.transpose(pT[:, :], a1[:, k * 128 : (k + 1) * 128], idn[:, :])
                        aT = wpool.tile([128, 128], bf, tag="aT", bufs=2)
                        nc.scalar.activation(
                            aT[:, :], pT[:, :], mybir.ActivationFunctionType.Copy
                        )
                        nc.tensor.matmul(
                            ps2[:, :], aT[:, :], w2sb[:, k * OUT : (k + 1) * OUT],
                            start=(k == 0), stop=(k == NK2 - 1),
                        )
                    g2e = wpool.tile([128, OUT], bf, tag="g2e", bufs=2)
                    nc.scalar.activation(
                        g2e[:, :], ps2[:, :], mybir.ActivationFunctionType.Copy,
                        scale=dnv[:, t : t + 1],
                    )
                    nc.sync.dma_start(
                        out=g2s[t * 128 : (t + 1) * 128, :],
                        in_=g2e[:, :].bitcast(f32),
                    )
                    if t == cA - 1:
                        nc.gpsimd.collective_compute(
                            "AllGather",
                            mybir.AluOpType.bypass,
                            ins=[g2s[0 : cA * 128, :].opt()],
                            outs=[g2fA[:, :].opt()],
                            replica_groups=[list(range(N_CORES))],
                        )
                    off_lo += kl
                    off_hi += kh
            nc.gpsimd.collective_compute(
                "AllGather",
                mybir.AluOpType.bypass,
                ins=[g2s[cA * 128 :, :].opt()],
                outs=[g2fB[:, :].opt()],
                replica_groups=[list(range(N_CORES))],
            )

            # ---- phase 3: layer-2 aggregation -> output ----
            for p, (ptiles, Llo, Lhi, KLp, KHp) in enumerate(pairs):
                c8lo, c8hi = Llo // 16, Lhi // 16
                ix2 = wpool.tile([128, C8PMAX], i16, tag="ix2", bufs=3)
                nc.sync.dma_start(
                    out=ix2[:, : c8lo + c8hi],
                    in_=idx_p[p * 128 : (p + 1) * 128, : c8lo + c8hi],
                )
                m2 = msg_tile("m2", PK2)
                paired_gather(m2, KLp, KHp, g2fA, g2fB, ix2, c8lo, c8hi, Llo, Lhi, PK2)

                off_lo = 0
                off_hi = KLp
                for t in ptiles:
                    kl, kh = KL[t], KH[t]
                    nb = kl + kh
                    d2 = wpool.tile([128, NBMAX], f32, tag="dl2", bufs=3)
                    nc.sync.dma_start(
                        out=d2[:, :nb], in_=dl_p[t * 128 : (t + 1) * 128, :nb]
                    )
                    gs2 = wpool.tile([128, PK2], f32, tag="gs2", bufs=3)
                    nc.sync.dma_start(
                        out=gs2[:, :], in_=g2s[t * 128 : (t + 1) * 128, :]
                    )
                    ps3 = ppool.tile([128, OUT], f32, tag="p2", bufs=2)
                    nc.tensor.matmul(ps3[:, :], diags[t][:, :], b2sb[:, :], start=True, stop=False)
                    for b in range(nb):
                        oh2 = wpool.tile([128, 128], bf, tag="oh", bufs=8)
                        nc.vector.tensor_scalar(
                            oh2[:, :], iot[:, :], d2[:, b : b + 1], None,
                            mybir.AluOpType.is_equal,
                        )
                        blk = m2[:, off_lo + b, :] if b < kl else m2[:, off_hi + (b - kl), :]
                        src = blk.bitcast(bf)
                        nc.tensor.matmul(ps3[:, :], oh2[:, :], src, start=False, stop=False)
                    nc.tensor.matmul(ps3[:, :], idn[:, :], gs2[:, :].bitcast(bf), start=False, stop=True)

                    of = wpool.tile([128, OUT], bf, tag="of", bufs=3)
                    nc.scalar.activation(
                        of[:, :], ps3[:, :], mybir.ActivationFunctionType.Copy,
                        scale=dnv[:, t : t + 1],
                    )
                    nc.sync.dma_start(out=out_p[t * 128 : (t + 1) * 128, :], in_=of[:, :])
                    off_lo += kl
                    off_hi += kh

    nc.compile()
    return nc


def _preprocess(x, edge_index, W1, b1, W2, b2, cfg):
    N, E = cfg["N"], cfg["E"]
    IN, HID, OUT = cfg["IN"], cfg["HID"], cfg["OUT"]
    T = cfg["T"]
    PC = T * 128
    NPAD = PC * N_CORES
    TT = T * N_CORES

    src = np.asarray(edge_index[0], dtype=np.int64)
    dst = np.asarray(edge_index[1], dtype=np.int64)

    indeg = np.bincount(dst, minlength=N)
    deg = indeg.astype(np.float32) + 1.0
    dinv = 1.0 / np.sqrt(deg)

    # ---- balanced node -> (tile, slot) assignment (LPT greedy) ----
    order = np.argsort(-indeg, kind="stable")
    heap = [(0, t, 0) for t in range(TT)]  # (load, tile, used)
    heapq.heapify(heap)
    row_of_node = np.empty(N, dtype=np.int64)
    for n in order:
        load, t, used = heapq.heappop(heap)
        row_of_node[n] = t * 128 + used
        used += 1
        if used < 128 and t * 128 + used < NPAD:
            heapq.heappush(heap, (load + int(indeg[n]), t, used))
    # note: NPAD - N pad slots simply remain unassigned

    # ---- chunk-major (lo/hi) global row mapping for the AllGather tables ----
    cA = min(20, (T + 1) // 2)  # AG-A split point (early exchange)
    cB = T - cA
    rows = np.arange(NPAD)
    r_core = rows // PC
    r_toff = (rows % PC) // 128
    r_slot = rows % 128
    r_lo = r_toff < cA
    val_of_row = np.where(
        r_lo,
        r_core * cA * 128 + r_toff * 128 + r_slot,
        r_core * cB * 128 + (r_toff - cA) * 128 + r_slot,
    )

    # ---- per-edge quantities (identical for both layers) ----
    srow = row_of_node[src]
    drow = row_of_node[dst]
    e_core = drow // PC
    e_toff = (drow % PC) // 128
    e_slot = drow % 128
    e_lo = r_lo[srow]
    e_val = val_of_row[srow]

    # ---- segment counts per (core, tile, half) ----
    segkey = (e_core * T + e_toff) * 2 + (~e_lo).astype(np.int64)
    cnt = np.bincount(segkey, minlength=TT * 2).reshape(N_CORES, T, 2)

    # per-tile static caps (max over cores, SPMD-uniform)
    def r16(v):
        return (int(v) + 15) // 16 * 16

    def r128(v):
        return (int(v) + 127) // 128 * 128

    maxlo = [int(cnt[:, t, 0].max()) for t in range(T)]
    maxhi = [int(cnt[:, t, 1].max()) for t in range(T)]
    KL = [math.ceil(r128(v) / 128) for v in maxlo]
    KH = [math.ceil(r128(v) / 128) for v in maxhi]
    NBMAX = max(kl + kh for kl, kh in zip(KL, KH))

    # gather pairs: two consecutive tiles share one lo + one hi gather.
    # First tile padded to full 128-blocks (block-aligns the second tile);
    # second tile's tail is exact-16 except in the first two pairs, whose
    # SBUF slots see their first write (unwritten rows must never exist).
    import os
    exact = os.environ.get("GNN_EXACT", "0") == "1"
    pairs = []  # (tiles, Llo, Lhi, KLp, KHp, cap_lo_list, cap_hi_list)
    for p0 in range(0, T, 2):
        ptiles = [t for t in (p0, p0 + 1) if t < T]
        ex = exact and len(pairs) >= 2
        caps_lo, caps_hi = [], []
        for j, t in enumerate(ptiles):
            if j == len(ptiles) - 1 and ex:
                caps_lo.append(r16(maxlo[t]))
                caps_hi.append(r16(maxhi[t]))
            else:
                caps_lo.append(KL[t] * 128)
                caps_hi.append(KH[t] * 128)
        pairs.append(
            (ptiles, sum(caps_lo), sum(caps_hi), sum(KL[t] for t in ptiles),
             sum(KH[t] for t in ptiles), caps_lo, caps_hi)
        )
    C8PMAX = max((pr[1] + pr[2]) // 16 for pr in pairs)

    # ---- per-core edge metadata (shared by both layers) ----
    idxt = np.zeros((N_CORES, len(pairs) * 128, C8PMAX), dtype=np.int16)
    dlt = np.full((N_CORES, T * 128, NBMAX), 999.0, dtype=np.float32)
    ordk = np.lexsort((srow, e_val, (~e_lo).astype(np.int64), e_toff, e_core))
    sv_s, ss_s = e_val[ordk], e_slot[ordk]
    segkey_s = segkey[ordk]
    # segment start offsets for every (core, tile, half)
    seg_start = np.searchsorted(segkey_s, np.arange(TT * 2))
    seg_end = np.searchsorted(segkey_s, np.arange(TT * 2) + 1)

    tile_of = {}
    for pi, pr in enumerate(pairs):
        for j, t in enumerate(pr[0]):
            tile_of[t] = (pi, j)

    for c in range(N_CORES):
        for pi, (ptiles, Llo, Lhi, KLp, KHp, caps_lo, caps_hi) in enumerate(pairs):
            col = 0
            for h, caps in ((0, caps_lo), (1, caps_hi)):
                li = []
                for j, t in enumerate(ptiles):
                    k = (c * T + t) * 2 + h
                    a, b = seg_start[k], seg_end[k]
                    n = b - a
                    cap = caps[j]
                    assert n <= cap
                    seg_idx = np.zeros(cap, dtype=np.int64)
                    seg_idx[:n] = sv_s[a:b]
                    li.append(seg_idx)
                    # dl for this tile/half
                    nblk = KL[t] if h == 0 else KH[t]
                    dll = np.full(nblk * 128, 999.0, dtype=np.float32)
                    dll[:n] = ss_s[a:b]
                    bb = 0 if h == 0 else KL[t]
                    dlt[c, t * 128 : (t + 1) * 128, bb : bb + nblk] = (
                        dll.reshape(nblk, 128).T
                    )
                cat = np.concatenate(li) if li else np.zeros(0, dtype=np.int64)
                if cat.shape[0]:
                    idxt[c, pi * 128 : (pi + 1) * 128, col : col + cat.shape[0] // 16] = (
                        _pack_idx(cat)
                    )
                col += cat.shape[0] // 16

    # ---- dense host tensors ----
    xPermT = np.zeros((IN, NPAD), dtype=np.float32)
    xPermT[:, row_of_node] = np.asarray(x, dtype=np.float32).T
    dinv_row = np.zeros(NPAD, dtype=np.float32)
    dinv_row[row_of_node] = dinv
    dsq_row = np.zeros(NPAD, dtype=np.float32)
    dsq_row[row_of_node] = np.sqrt(deg)

    NK1, NK2 = IN // 128, HID // 128
    w1p = (
        np.asarray(W1, np.float32).reshape(NK1, 128, HID).transpose(1, 0, 2).reshape(128, NK1 * HID).astype(BF16)
    )
    w2p = (
        np.asarray(W2, np.float32).reshape(NK2, 128, OUT).transpose(1, 0, 2).reshape(128, NK2 * OUT).astype(BF16)
    )
    b1r = np.tile(np.asarray(b1, np.float32)[None, :], (128, 1)).astype(BF16)
    b2r = np.tile(np.asarray(b2, np.float32)[None, :], (128, 1)).astype(BF16)
    iota = np.tile(np.arange(128, dtype=np.float32)[None, :], (128, 1)).astype(BF16)
    identb = np.eye(128, dtype=np.float32).astype(BF16)

    in_maps = []
    for c in range(N_CORES):
        sl_c = slice(c * PC, (c + 1) * PC)
        in_maps.append(
            {
                "xT": xPermT[:, sl_c].astype(BF16),
                "w1p": w1p,
                "w2p": w2p,
                "b1r": b1r,
                "b2r": b2r,
                "iota": iota,
                "identb": identb,
                "dinvT": dinv_row[sl_c].reshape(T, 128).T.astype(np.float32).copy(),
                "dsqT": dsq_row[sl_c].reshape(T, 128).T.astype(np.float32).copy(),
                "idxt": idxt[c],
                "dlt": dlt[c],
            }
        )

    meta = dict(
        ch_tiles=[cA, cB],
        KL=KL, KH=KH,
        pairs=[pr[:5] for pr in pairs],
        NBMAX=NBMAX, C8PMAX=C8PMAX,
        row_of_node=row_of_node,
    )
    return in_maps, meta


def kernel(x, edge_index, W1, b1, W2, b2):
    cfg = _CFG
    N, OUT = cfg["N"], cfg["OUT"]
    PC = cfg["T"] * 128
    in_maps, meta = _preprocess(x, edge_index, W1, b1, W2, b2, cfg)
    nc = _build_nc(cfg, meta)
    import os
    if os.environ.get("GNN_SIM"):
        from concourse import bass_interp

        sim = bass_interp.MultiCoreSim(nc, N_CORES)
        for c in range(N_CORES):
            for k, v in in_maps[c].items():
                sim.cores[c].tensor(k)[:] = v
        sim.simulate()
        results = [
            {"out": np.array(sim.cores[c].tensor("out"))} for c in range(N_CORES)
        ]
    else:
        res = run_bass_kernel_spmd(nc, in_maps, core_ids=list(range(N_CORES)))
        results = res.results
    out = np.empty((N, OUT), dtype=np.float32)
    row = meta["row_of_node"]
    core = row // PC
    local = row % PC
    for c in range(N_CORES):
        m = core == c
        out[np.where(m)[0]] = results[c]["out"][local[m]].astype(np.float32)
    return out
